# revision 1
# baseline (speedup 1.0000x reference)
"""Trainium2 Bass kernel for causal self-attention with cumulative-phase rotary
embedding (nn_CausalSelfAttention_64338610094602).

Sharding: 8 cores = 4 batches x 2 head-groups (tensor-parallel over heads).
Each core computes, for its (batch, 8-head group):
  omega/phi (replicated per batch), QKV projections, rotation + RMSNorm,
  causal attention (transposed-scores layout, max-free softmax), and a
  partial output projection. Host sums the two head-group partials per batch.

All big GEMMs run in float32r (full PE rate at N>=256, ~13-bit mantissa).
The phase/cumsum/trig path is kept in fp32.
"""
import math

import numpy as np
import ml_dtypes

import concourse.mybir as mybir
import concourse.tile as tile
from concourse import bacc
from concourse.bass_utils import run_bass_kernel_spmd

B, T, C = 4, 2048, 2048
H, D, DH = 16, 128, 64
HG = 8          # heads per core (head-group)
GD = HG * D     # group output dims = 1024
NT = T // 512   # 4 t-blocks of 512
NCT = C // 128  # 16 contraction tiles
EPS = 1e-5
SCL = 1.0 / math.sqrt(D)

dt = mybir.dt
AF = mybir.ActivationFunctionType
ALU = mybir.AluOpType

TWO_PI = 6.283185307179586
INV_2PI = 1.0 / TWO_PI
CW1 = float(np.float32(6.28125))
CW2 = float(np.float32(TWO_PI - 6.28125))
CW3 = float(TWO_PI - CW1 - float(np.float32(TWO_PI - 6.28125)))
MAGIC = 12582912.0  # 1.5 * 2^23: fp32 add/sub rounds to nearest int
HALF_PI = 1.5707963267948966
PI = 3.141592653589793

_CACHE = {}


def _round_f32r(x):
    """Round fp32 array to float32r (13-bit mantissa, round-to-nearest-even)."""
    x = np.ascontiguousarray(x, dtype=np.float32)
    b = x.view(np.uint32).copy()
    low = b & np.uint32(0x3FF)
    bb = b & ~np.uint32(0x3FF)
    rnd = (low > 0x200) | ((low == 0x200) & (((bb >> 10) & 1) == 1))
    return (bb + (rnd.astype(np.uint32) << 10)).view(np.float32)


def _build():
    f32, f32r, bf16 = dt.float32, dt.float32r, dt.bfloat16
    nc = bacc.Bacc(None, target_bir_lowering=False)
    with tile.TileContext(nc) as tc:
        xt_d = nc.dram_tensor("xt", (C, T), f32r, kind="ExternalInput")
        wq_d = nc.dram_tensor("wq", (C, GD), f32r, kind="ExternalInput")
        wk_d = nc.dram_tensor("wk", (C, GD), f32r, kind="ExternalInput")
        wv_d = nc.dram_tensor("wv", (C, GD), f32r, kind="ExternalInput")
        wo_d = nc.dram_tensor("wo", (GD, C), f32r, kind="ExternalInput")
        womg_d = nc.dram_tensor("womg", (128, NCT), f32r, kind="ExternalInput")
        b16_d = nc.dram_tensor("b16", (1, 1), f32, kind="ExternalInput")
        logf_d = nc.dram_tensor("logf", (DH, 1), f32, kind="ExternalInput")
        gq_d = nc.dram_tensor("gq", (128, 1), f32, kind="ExternalInput")
        gk_d = nc.dram_tensor("gk", (128, 1), f32, kind="ExternalInput")
        masks_d = nc.dram_tensor("masks", (128, 4 * 512), bf16, kind="ExternalInput")
        onesA_d = nc.dram_tensor("onesA", (128, 1), f32r, kind="ExternalInput")
        onesB_d = nc.dram_tensor("onesB", (1, 128), f32r, kind="ExternalInput")
        ones64_d = nc.dram_tensor("ones64", (1, DH), f32, kind="ExternalInput")
        oneh31_d = nc.dram_tensor("oneh31", (128, 31), f32r, kind="ExternalInput")
        out_d = nc.dram_tensor("out", (T, C), f32, kind="ExternalOutput")

        with tc.tile_pool(name="dram", bufs=1, space="DRAM") as dramp:
            yspill = dramp.tile([128, HG * T], f32r)  # yT per head at col h*T

            with tc.tile_pool(name="const", bufs=1) as constp:
                womg = constp.tile([128, NCT], f32r)
                nc.sync.dma_start(womg[:], womg_d[:])
                b16t = constp.tile([1, 1], f32)
                nc.sync.dma_start(b16t[:], b16_d[:])
                logf = constp.tile([DH, 1], f32)
                nc.sync.dma_start(logf[:], logf_d[:])
                gqt = constp.tile([128, 1], f32)
                nc.sync.dma_start(gqt[:], gq_d[:])
                gkt = constp.tile([128, 1], f32)
                nc.sync.dma_start(gkt[:], gk_d[:])
                onesA = constp.tile([128, 1], f32r)
                nc.sync.dma_start(onesA[:], onesA_d[:])
                onesB = constp.tile([1, 128], f32r)
                nc.sync.dma_start(onesB[:], onesB_d[:])
                ones64 = constp.tile([1, DH], f32)
                nc.sync.dma_start(ones64[:], ones64_d[:])
                epst = constp.tile([1, 1], f32)
                nc.vector.memset(epst[:], EPS)
                eps16 = constp.tile([16, 1], f32)
                nc.vector.memset(eps16[:], EPS)
                oneh31 = constp.tile([128, 31], f32r)
                nc.sync.dma_start(oneh31[:], oneh31_d[:])
                freq = constp.tile([DH, 1], f32)
                nc.scalar.activation(freq[:], logf[:], AF.Exp)

                _main(nc, tc, xt_d, wq_d, wk_d, wv_d, masks_d, yspill,
                      womg, b16t, gqt, gkt, onesA, onesB, ones64, eps16,
                      oneh31, freq)

                # ---- P3: output projection out = yall^T @ wo ----
                with tc.tile_pool(name="p3", bufs=1) as p3, \
                     tc.tile_pool(name="p3o", bufs=3) as p3o, \
                     tc.tile_pool(name="p3ps", bufs=4, space="PSUM") as p3ps:
                    yall = p3.tile([128, HG * T], f32r)
                    wosb = p3.tile([128, HG * C], f32r)
                    for h in range(HG):
                        nc.sync.dma_start(yall[:, h * T:(h + 1) * T],
                                          yspill[:, h * T:(h + 1) * T])
                        nc.sync.dma_start(wosb[:, h * C:(h + 1) * C],
                                          wo_d[h * 128:(h + 1) * 128, :])
                    for ti in range(T // 128):
                        for cb in range(C // 512):
                            ops = p3ps.tile([128, 512], f32, tag="o")
                            for h in range(HG):
                                nc.tensor.matmul(
                                    ops[:],
                                    yall[:, h * T + ti * 128:h * T + (ti + 1) * 128],
                                    wosb[:, h * C + cb * 512:h * C + (cb + 1) * 512],
                                    start=(h == 0), stop=(h == HG - 1))
                            osb = p3o.tile([128, 512], f32, tag="osb")
                            nc.vector.tensor_copy(osb[:], ops[:])
                            nc.sync.dma_start(
                                out_d[ti * 128:(ti + 1) * 128, cb * 512:(cb + 1) * 512],
                                osb[:])
    nc.compile()
    return nc


def _main(nc, tc, xt_d, wq_d, wk_d, wv_d, masks_d, yspill,
          womg, b16t, gqt, gkt, onesA, onesB, ones64, eps16, oneh31, freq):
    f32, f32r, bf16 = dt.float32, dt.float32r, dt.bfloat16

    with tc.tile_pool(name="big", bufs=1) as bigp, \
         tc.tile_pool(name="xtp", bufs=1) as xtp:
        trig = bigp.tile([128, T], f32)       # [0:64]=cos, [64:128]=sin
        masks = bigp.tile([128, 4 * 512], bf16)
        nc.sync.dma_start(masks[:], masks_d[:])

        xts = xtp.tile([128, NCT * T], f32r)  # c-tile i at cols [i*T, (i+1)*T)
        for i in range(NCT):
            nc.sync.dma_start(xts[:, i * T:(i + 1) * T],
                              xt_d[i * 128:(i + 1) * 128, :])

        # ---- P1: omega -> phi -> trig ----
        with tc.tile_pool(name="p1", bufs=1) as p1, \
             tc.tile_pool(name="p1b", bufs=2) as p1b, \
             tc.tile_pool(name="p1ps", bufs=2, space="PSUM") as p1ps:
            omega = p1.tile([1, T], f32)
            for J in range(NT):
                omps = p1ps.tile([1, 512], f32, tag="om")
                for i in range(NCT):
                    nc.tensor.matmul(
                        omps[:], womg[:, i:i + 1],
                        xts[:, i * T + J * 512:i * T + J * 512 + 512],
                        start=(i == 0), stop=(i == NCT - 1))
                nc.scalar.activation(omega[:, J * 512:(J + 1) * 512], omps[:],
                                     AF.Sigmoid, scale=1.0 / 16.0, bias=b16t[:])
            incl = p1.tile([1, T], f32)
            nc.vector.tensor_tensor_scan(incl[:], omega[:], omega[:], 0.0,
                                         ALU.add, ALU.bypass)
            phi = p1.tile([1, T], f32)
            nc.vector.tensor_sub(phi[:], incl[:], omega[:])
            for J in range(NT):
                sl = slice(J * 512, (J + 1) * 512)
                phps = p1ps.tile([DH, 512], f32, tag="phib")
                nc.tensor.matmul(phps[:], ones64[:], phi[:, sl],
                                 start=True, stop=True)
                ang = p1b.tile([DH, 512], f32, tag="ang")
                nc.vector.tensor_scalar(ang[:], phps[:], freq[:], None, op0=ALU.mult)
                mm = p1b.tile([DH, 512], f32, tag="mm")
                nc.vector.tensor_scalar(mm[:], ang[:], INV_2PI, MAGIC,
                                        op0=ALU.mult, op1=ALU.add)
                kk = p1b.tile([DH, 512], f32, tag="kk")
                nc.vector.tensor_scalar_add(kk[:], mm[:], -MAGIC)
                red = p1b.tile([DH, 512], f32, tag="red")
                nc.vector.cody_waite_cascade(red[:], ang[:], kk[:], CW1, CW2, CW3)
                red2 = p1b.tile([DH, 512], f32, tag="red2")
                nc.vector.add_range_wrap(red2[:], red[:], HALF_PI, PI, TWO_PI)
                nc.scalar.activation(trig[0:DH, sl], red2[:], AF.Sin)   # cos
                nc.scalar.activation(trig[DH:128, sl], red[:], AF.Sin)  # sin

        # ---- P2: per head-pair: QKV + rot/norm + attention ----
        with tc.tile_pool(name="qkv", bufs=1) as qkvp, \
             tc.tile_pool(name="wst", bufs=3) as wst, \
             tc.tile_pool(name="sc512", bufs=1) as sc512, \
             tc.tile_pool(name="rows", bufs=1) as rowsp:
            for pair in range(4):
                q_sb = qkvp.tile([128, 2 * T], f32r, tag="q", name=f"q_{pair}")
                k_sb = qkvp.tile([128, 2 * T], f32r, tag="k", name=f"k_{pair}")
                v_sb = qkvp.tile([128, 16 * 256], f32r, tag="v", name=f"v_{pair}")

                # --- 2a: q/k for both heads (4 J banks); ssq comes straight
                # from the pre-rotation tile (rotation is norm-preserving);
                # gamma rides the ACT eviction copy; rstd batched per pair. ---
                with tc.tile_pool(name=f"psA_{pair}", bufs=1, space="PSUM") as psA, \
                     tc.tile_pool(name=f"psS_{pair}", bufs=1, space="PSUM") as psS, \
                     tc.tile_pool(name=f"psR_{pair}", bufs=2, space="PSUM") as psR:
                    ssqps = psS.tile([16, 512], f32, tag="ssq",
                                     name=f"ssqps_{pair}")
                    site = 0
                    for wi, (w_d, gam, dest) in enumerate(
                            ((wq_d, gqt, q_sb), (wk_d, gkt, k_sb))):
                        for hl in range(2):
                            h = pair * 2 + hl
                            qps = {}
                            for J in range(NT):
                                qp = psA.tile([128, 512], f32, tag=f"q{J}",
                                              name=f"qp_{pair}_{wi}_{hl}_{J}")
                                qps[J] = qp
                            for i in range(NCT):
                                wt = wst.tile([128, 128], f32r, tag="w")
                                nc.sync.dma_start(
                                    wt[:],
                                    w_d[i * 128:(i + 1) * 128,
                                        h * 128:(h + 1) * 128])
                                for J in range(NT):
                                    nc.tensor.matmul(
                                        qps[J][:], wt[:],
                                        xts[:, i * T + J * 512:i * T + J * 512 + 512],
                                        start=(i == 0), stop=(i == NCT - 1))
                            for J in range(NT):
                                rot = _rotate(nc, sc512, qps[J], trig, J)
                                sq = sc512.tile([128, 512], f32r, tag="ta",
                                                name=f"sq_{pair}_{site}")
                                nc.scalar.activation(sq[:], qps[J][:], AF.Square)
                                nc.tensor.matmul(
                                    ssqps[:], oneh31[:, 15 - site:31 - site],
                                    sq[:],
                                    start=(site == 0), stop=(site == 15))
                                dcol = hl * T + J * 512
                                nc.scalar.activation(
                                    dest[:, dcol:dcol + 512], rot[:], AF.Copy,
                                    scale=gam[:])
                                site += 1
                    # batched rstd = exp(-0.5 * ln(ssq/128 + eps)) for 16 sites
                    lnt = sc512.tile([16, 512], f32, tag="ta",
                                     name=f"lnt_{pair}")
                    nc.scalar.activation(lnt[:], ssqps[:], AF.Ln,
                                         scale=1.0 / 128.0, bias=eps16[:])
                    rstd = sc512.tile([16, 512], f32r, tag="tb",
                                      name=f"rstd_{pair}")
                    nc.scalar.activation(rstd[:], lnt[:], AF.Exp, scale=-0.5)
                # --- 2b: v for both heads (N=256 wide) ---
                with tc.tile_pool(name=f"ps2b_{pair}", bufs=1, space="PSUM") as psb:
                    for half in range(2):
                        vps = []
                        for t in range(8):
                            vp = psb.tile([128, 256], f32, tag=f"v{t}",
                                          name=f"vp_{pair}_{half}_{t}")
                            vps.append(vp)
                        for i in range(NCT):
                            wvt = wst.tile([128, 256], f32r, tag="wv")
                            nc.sync.dma_start(
                                wvt[:],
                                wv_d[i * 128:(i + 1) * 128,
                                     pair * 256:(pair + 1) * 256])
                            for t in range(8):
                                tt = half * 8 + t
                                nc.tensor.matmul(
                                    vps[t][:],
                                    xts[:, i * T + tt * 128:i * T + (tt + 1) * 128],
                                    wvt[:],
                                    start=(i == 0), stop=(i == NCT - 1))
                        for t in range(8):
                            tt = half * 8 + t
                            nc.vector.tensor_copy(
                                v_sb[:, tt * 256:(tt + 1) * 256], vps[t][:])

                # --- deferred q/k normalize (rstd broadcast): runs after
                # the v matmuls so the PE never waits on the Ln/Exp chain ---
                with tc.tile_pool(name=f"psN_{pair}", bufs=2, space="PSUM") as psR:
                    site = 0
                    for wi in range(2):
                        dest = (q_sb, k_sb)[wi]
                        for hl in range(2):
                            for J in range(NT):
                                rrow = rowsp.tile([1, 512], f32r,
                                                  tag="r1" if site % 2 == 0 else "r0",
                                                  name=f"rrow_{pair}_{site}")
                                nc.sync.dma_start(rrow[:], rstd[site:site + 1, :])
                                rbps = psR.tile([128, 512], f32, tag="rb",
                                                name=f"rb2a_{pair}_{site}")
                                nc.tensor.matmul(rbps[:], onesB[:], rrow[:],
                                                 start=True, stop=True)
                                dcol = hl * T + J * 512
                                nc.vector.tensor_tensor(
                                    dest[:, dcol:dcol + 512],
                                    dest[:, dcol:dcol + 512], rbps[:],
                                    op=ALU.mult)
                                site += 1


                # --- 2c: attention per head; each J's softmax epilogue is
                # emitted inside the next J's matmul stream so the in-order
                # PE queue never stalls on the recip -> broadcast chain ---
                with tc.tile_pool(name=f"ps2c_{pair}", bufs=2, space="PSUM") as psc:
                    pend = [None]

                    def epilogue(yps, dps, hl, J):
                        h = pair * 2 + hl
                        rcf = rowsp.tile([1, 512], f32, tag="r0",
                                         name=f"rcf_{pair}_{hl}_{J}")
                        nc.vector.reciprocal_approx_fast(out=rcf[:], in_=dps[:])
                        recip = rowsp.tile([1, 512], f32r, tag="r1",
                                           name=f"recip_{pair}_{hl}_{J}")
                        nc.vector.tensor_copy(recip[:], rcf[:])
                        rbps = psc.tile([128, 512], f32, tag="rb", bufs=1,
                                        name=f"rbps_{pair}_{hl}_{J}")
                        nc.tensor.matmul(rbps[:], onesB[:], recip[:],
                                         start=True, stop=True)
                        rbsb = sc512.tile([128, 512], f32, tag="tb",
                                          name=f"rbsb_{pair}_{hl}_{J}")
                        nc.scalar.copy(rbsb[:], rbps[:])
                        yt = sc512.tile([128, 512], f32r, tag="ta",
                                        name=f"yt_{pair}_{hl}_{J}")
                        nc.vector.tensor_tensor(yt[:], yps[:], rbsb[:],
                                                op=ALU.mult)
                        nc.sync.dma_start(
                            yspill[:, h * T + J * 512:h * T + (J + 1) * 512],
                            yt[:])

                    for hl in range(2):
                        for J in range(NT):
                            nI = 4 * J + 4
                            yps = psc.tile([128, 512], f32, tag="y",
                                           name=f"yps_{pair}_{hl}_{J}")
                            dps = psc.tile([1, 512], f32, tag="den",
                                           name=f"dps_{pair}_{hl}_{J}")
                            for I in range(nI):
                                sps = psc.tile([128, 512], f32, tag="s", bufs=3,
                                               name=f"sps_{pair}_{hl}_{J}_{I}")
                                nc.tensor.matmul(
                                    sps[:],
                                    k_sb[:, hl * T + I * 128:hl * T + (I + 1) * 128],
                                    q_sb[:, hl * T + J * 512:hl * T + (J + 1) * 512],
                                    start=True, stop=True)
                                ex = sc512.tile([128, 512], f32r,
                                                tag="ex" if I % 2 == 0 else "ex2",
                                                name=f"ex_{pair}_{hl}_{J}_{I}")
                                nc.scalar.activation(ex[:], sps[:], AF.Exp, scale=SCL)
                                if I >= 4 * J:
                                    r = I - 4 * J
                                    exm = sc512.tile([128, 512], f32r, tag="rot",
                                                     name=f"exm_{pair}_{hl}_{J}_{I}")
                                    nc.vector.tensor_tensor(
                                        exm[:], ex[:], masks[:, r * 512:(r + 1) * 512],
                                        op=ALU.mult)
                                    use = exm
                                else:
                                    use = ex
                                nc.tensor.matmul(
                                    yps[:],
                                    v_sb[:, I * 256 + hl * 128:I * 256 + hl * 128 + 128],
                                    use[:], start=(I == 0), stop=(I == nI - 1))
                                nc.tensor.matmul(
                                    dps[:], onesA[:], use[:],
                                    start=(I == 0), stop=(I == nI - 1))
                                if I == 1 and pend[0] is not None:
                                    pend[0]()
                                    pend[0] = None
                            pend[0] = (lambda yps=yps, dps=dps, hl=hl, J=J:
                                       epilogue(yps, dps, hl, J))
                    pend[0]()
                    pend[0] = None


def _rotate(nc, sc512, qps, trig, J):
    """Rotate (cumulative-phase RoPE) one (128, 512) projection PSUM tile.

    trig[0:64]=cos, [64:128]=sin for this J. Returns the rotated f32 tile.
    Ordered so the PSUM bank is released after the first 3 DVE ops."""
    f32 = dt.float32
    sl = slice(J * 512, (J + 1) * 512)
    ta = sc512.tile([DH, 512], f32, tag="ta")      # q1*cos
    tb = sc512.tile([DH, 512], f32, tag="tb")      # q2*sin
    tcc = sc512.tile([DH, 512], f32, tag="ex")     # q2*cos (ex slot: 2c-only)
    td = sc512.tile([DH, 512], f32, tag="ex2")     # q1*sin (ex2 slot: 2c-only)
    rot = sc512.tile([128, 512], f32, tag="rot")
    nc.vector.tensor_tensor(ta[:], qps[0:DH, :], trig[0:DH, sl], op=ALU.mult)
    nc.vector.tensor_tensor(tb[:], qps[DH:128, :], trig[DH:128, sl], op=ALU.mult)
    nc.vector.tensor_tensor(tcc[:], qps[DH:128, :], trig[0:DH, sl], op=ALU.mult)
    nc.vector.tensor_tensor(td[:], qps[0:DH, :], trig[DH:128, sl], op=ALU.mult)
    # PSUM bank free from here on
    nc.vector.tensor_add(rot[0:DH, :], ta[:], tb[:])
    nc.vector.tensor_sub(rot[DH:128, :], tcc[:], td[:])
    return rot


def _host_prep(inputs):
    x = np.asarray(inputs["x"], dtype=np.float32)
    Wq = np.asarray(inputs["Wq"], dtype=np.float32)
    Wk = np.asarray(inputs["Wk"], dtype=np.float32)
    Wv = np.asarray(inputs["Wv"], dtype=np.float32)
    Wo = np.asarray(inputs["Wo"], dtype=np.float32)
    w_omega = np.asarray(inputs["w_omega"], dtype=np.float32)
    b_omega = np.asarray(inputs["b_omega"], dtype=np.float32)
    log_freq = np.asarray(inputs["log_freq"], dtype=np.float32)
    q_gamma = np.asarray(inputs["q_gamma"], dtype=np.float32)
    k_gamma = np.asarray(inputs["k_gamma"], dtype=np.float32)

    womg = _round_f32r(w_omega.reshape(NCT, 128).T)  # [p, i] = w_omega[i*128+p]
    b16 = (b_omega / 16.0).reshape(1, 1).astype(np.float32)
    logf = log_freq.reshape(DH, 1)
    gq = q_gamma.reshape(128, 1)
    gk = k_gamma.reshape(128, 1)
    p = np.arange(128)[:, None]
    c = np.arange(512)[None, :]
    masks = np.concatenate(
        [((p + r * 128) <= c).astype(np.float32) for r in range(4)], axis=1
    ).astype(ml_dtypes.bfloat16)
    onesA = np.ones((128, 1), dtype=np.float32)
    onesB = np.ones((1, 128), dtype=np.float32)
    ones64 = np.ones((1, DH), dtype=np.float32)
    oneh31 = np.zeros((128, 31), dtype=np.float32)
    oneh31[:, 15] = 1.0

    in_maps = []
    for core in range(8):
        b, g = core // 2, core % 2
        in_maps.append({
            "xt": _round_f32r(x[b].T),
            "wq": _round_f32r(Wq[g * GD:(g + 1) * GD, :].T),
            "wk": _round_f32r(Wk[g * GD:(g + 1) * GD, :].T),
            "wv": _round_f32r(Wv[g * GD:(g + 1) * GD, :].T),
            "wo": _round_f32r(Wo[:, g * GD:(g + 1) * GD].T),
            "womg": womg, "b16": b16, "logf": logf, "gq": gq, "gk": gk,
            "masks": masks, "onesA": onesA, "onesB": onesB, "ones64": ones64,
            "oneh31": oneh31,
        })
    return in_maps


def kernel(**inputs) -> np.ndarray:
    if "nc" not in _CACHE:
        _CACHE["nc"] = _build()
    nc = _CACHE["nc"]
    in_maps = _host_prep(inputs)
    res = run_bass_kernel_spmd(nc, in_maps, core_ids=list(range(8)))
    out = np.empty((B, T, C), dtype=np.float32)
    for b in range(B):
        out[b] = res.results[2 * b]["out"] + res.results[2 * b + 1]["out"]
    return out



# revision 30
# speedup vs baseline: 1.2617x; 1.2617x over previous
"""Trainium2 Bass kernel for causal self-attention with cumulative-phase rotary
embedding (nn_CausalSelfAttention_64338610094602).

Sharding: 8 cores = 4 batches x 2 head-groups (tensor-parallel over heads).
Each core computes, for its (batch, 8-head group), per head:
  QKV projections, rotation + RMSNorm, causal attention (transposed-scores
  layout, max-free softmax), and a partial output projection. Host sums the
  two head-group partials per batch.

The tiny omega/cumsum/trig path is computed on host (0.016% of FLOPs) and
shipped as cos/sin tables. Score-path GEMMs run in float32r; the probs/V/Wo
side runs in bf16 (rel-err budget ~5e-3 vs 2e-2 tolerance).

Key scheduling ideas vs the previous version:
  - PSUM projection banks are released by a single Pool-engine eviction copy;
    rotation (4 fused DVE ops via duplicated cos/sin tables) reads the SBUF
    copy off the critical path and writes q/k tiles directly (f32r cast).
  - k-side rstd (and the 1/sqrt(D) scale, and both gammas via host-folded
    Wk) ride the exp's per-partition scale, via a PE-transposed rstd column
    table; only the q side needs broadcast-normalize.
  - softmax is software-pipelined 2 blocks deep; denominators are packed
    pairwise (DVE bf16 adds) to halve the PE's ones-matmuls.
  - V is computed d-major at full moving width then PE-transposed to
    token-major bf16.
  - P3 reloads y/Wo (bf16) with DMA prefetched under the last head's
    attention, after the x^T buffer is freed.
"""
import math

import numpy as np
import ml_dtypes

import concourse.mybir as mybir
import concourse.tile as tile
from concourse import bacc
from concourse.bass_utils import run_bass_kernel_spmd

B, T, C = 4, 2048, 2048
H, D, DH = 16, 128, 64
HG = 8          # heads per core (head-group)
GD = HG * D     # group output dims = 1024
NT = T // 512   # 4 J-blocks of 512
NCT = C // 128  # 16 contraction tiles
EPS = 1e-5
SCL = 1.0 / math.sqrt(D)
OMEGA_SCALE = 16.0

dt = mybir.dt
AF = mybir.ActivationFunctionType
ALU = mybir.AluOpType

_CACHE = {}


def _round_f32r(x):
    """Round fp32 array to float32r (13-bit mantissa, round-to-nearest-even)."""
    x = np.ascontiguousarray(x, dtype=np.float32)
    b = x.view(np.uint32).copy()
    low = b & np.uint32(0x3FF)
    bb = b & ~np.uint32(0x3FF)
    rnd = (low > 0x200) | ((low == 0x200) & (((bb >> 10) & 1) == 1))
    return (bb + (rnd.astype(np.uint32) << 10)).view(np.float32)


def _build():
    f32, f32r, bf16 = dt.float32, dt.float32r, dt.bfloat16
    nc = bacc.Bacc(None, target_bir_lowering=False)
    with tile.TileContext(nc) as tc:
        xt_d = nc.dram_tensor("xt", (C, T), f32r, kind="ExternalInput")
        wq_d = nc.dram_tensor("wq", (C, GD), f32r, kind="ExternalInput")
        wk_d = nc.dram_tensor("wk", (C, GD), f32r, kind="ExternalInput")
        wv_d = nc.dram_tensor("wv", (C, GD), f32r, kind="ExternalInput")
        wo_d = nc.dram_tensor("wo", (GD, C), bf16, kind="ExternalInput")
        trigc_d = nc.dram_tensor("trigc", (128, T), f32, kind="ExternalInput")
        trigs_d = nc.dram_tensor("trigs", (128, T), f32, kind="ExternalInput")
        masks_d = nc.dram_tensor("masks", (128, 4 * 512), bf16, kind="ExternalInput")
        onesAb_d = nc.dram_tensor("onesAb", (128, 1), bf16, kind="ExternalInput")
        onesB_d = nc.dram_tensor("onesB", (1, 128), f32r, kind="ExternalInput")
        idb_d = nc.dram_tensor("idb", (128, 128), bf16, kind="ExternalInput")
        idr_d = nc.dram_tensor("idr", (128, 128), f32r, kind="ExternalInput")
        wcolq_d = nc.dram_tensor("wcolq", (128, 15), f32r, kind="ExternalInput")
        wcolk_d = nc.dram_tensor("wcolk", (128, 15), f32r, kind="ExternalInput")
        bias8_d = nc.dram_tensor("bias8", (8, 1), f32, kind="ExternalInput")
        out_d = nc.dram_tensor("out", (T, C), f32, kind="ExternalOutput")

        with tc.tile_pool(name="dram", bufs=1, space="DRAM") as dramp:
            yspill = dramp.tile([128, 7 * T], bf16)  # heads 0-6 yT

            with tc.tile_pool(name="const", bufs=1) as constp:
                trigc = constp.tile([128, T], f32)
                nc.sync.dma_start(trigc[:], trigc_d[:])
                trigs = constp.tile([128, T], f32)
                nc.sync.dma_start(trigs[:], trigs_d[:])
                masks = constp.tile([128, 4 * 512], bf16)
                nc.sync.dma_start(masks[:], masks_d[:])
                onesAb = constp.tile([128, 1], bf16)
                nc.sync.dma_start(onesAb[:], onesAb_d[:])
                onesB = constp.tile([1, 128], f32r)
                nc.sync.dma_start(onesB[:], onesB_d[:])
                idb = constp.tile([128, 128], bf16)
                nc.sync.dma_start(idb[:], idb_d[:])
                idr = constp.tile([128, 128], f32r)
                nc.sync.dma_start(idr[:], idr_d[:])
                wcolq = constp.tile([128, 15], f32r)
                nc.sync.dma_start(wcolq[:], wcolq_d[:])
                wcolk = constp.tile([128, 15], f32r)
                nc.sync.dma_start(wcolk[:], wcolk_d[:])
                bias8 = constp.tile([8, 1], f32)
                nc.sync.dma_start(bias8[:], bias8_d[:])
                eps8 = constp.tile([8, 1], f32)
                nc.vector.memset(eps8[:], EPS)

                with tc.tile_pool(name="qkv", bufs=1) as qkvp, \
                     tc.tile_pool(name="wst", bufs=3) as wst, \
                     tc.tile_pool(name="wk512", bufs=1) as wkp, \
                     tc.tile_pool(name="exp", bufs=1) as expool, \
                     tc.tile_pool(name="rows", bufs=1) as rowsp:
                    env = dict(
                        nc=nc, tc=tc, wst=wst, wkp=wkp, expool=expool,
                        rowsp=rowsp, qkvp=qkvp, trigc=trigc, trigs=trigs,
                        masks=masks, onesAb=onesAb, onesB=onesB, idb=idb,
                        idr=idr, wcolq=wcolq, wcolk=wcolk, bias8=bias8,
                        eps8=eps8, wq_d=wq_d, wk_d=wk_d, wv_d=wv_d,
                        yspill=yspill)

                    with tc.tile_pool(name="xtp", bufs=1) as xtp:
                        xts = xtp.tile([128, NCT * T], f32r)
                        for i in range(NCT):
                            nc.sync.dma_start(xts[:, i * T:(i + 1) * T],
                                              xt_d[i * 128:(i + 1) * 128, :])
                        env["xts"] = xts
                        heads = {}
                        for h in range(HG):
                            heads[h] = _head_qkv(env, h)
                            if h < HG - 1:
                                _head_attn(env, h, heads[h], None)
                    # xts freed; prefetch P3 operands under head 7's attention
                    with tc.tile_pool(name="p3", bufs=1) as p3p, \
                         tc.tile_pool(name="p3o", bufs=1) as p3o:
                        yall = p3p.tile([128, HG * T], bf16)
                        wosb = p3p.tile([128, HG * C], bf16)
                        for h in range(7):
                            nc.sync.dma_start(yall[:, h * T:(h + 1) * T],
                                              yspill[:, h * T:(h + 1) * T])
                        for h in range(HG):
                            nc.sync.dma_start(wosb[:, h * C:(h + 1) * C],
                                              wo_d[h * 128:(h + 1) * 128, :])
                        _head_attn(env, 7, heads[7], yall)

                        with tc.tile_pool(name="p3ps", bufs=1,
                                          space="PSUM") as p3ps:
                            for ti in range(T // 128):
                                ops = [p3ps.tile([128, 512], f32, tag=f"c{cb}",
                                                 bufs=2,
                                                 name=f"op_{ti}_{cb}")
                                       for cb in range(4)]
                                for hh in range(HG):
                                    for cb in range(4):
                                        nc.tensor.matmul(
                                            ops[cb][:],
                                            yall[:, hh * T + ti * 128:
                                                 hh * T + (ti + 1) * 128],
                                            wosb[:, hh * C + cb * 512:
                                                 hh * C + (cb + 1) * 512],
                                            start=(hh == 0), stop=(hh == HG - 1))
                                for cb in range(4):
                                    osb = p3o.tile([128, 512], f32,
                                                   tag=f"osb{cb % 2}",
                                                   bufs=2,
                                                   name=f"osb_{ti}_{cb}")
                                    if cb % 2:
                                        nc.vector.tensor_copy(osb[:], ops[cb][:])
                                    else:
                                        nc.scalar.copy(osb[:], ops[cb][:])
                                    nc.sync.dma_start(
                                        out_d[ti * 128:(ti + 1) * 128,
                                              cb * 512:(cb + 1) * 512],
                                        osb[:])
    nc.compile()
    return nc


def _head_qkv(env, h):
    """2b (V via d-major + PE transpose) and 2a (Q/K proj + rotate + rstd)."""
    nc = env["nc"]
    tc = env["tc"]
    xts, wst, wkp, qkvp = env["xts"], env["wst"], env["wkp"], env["qkvp"]
    f32, f32r, bf16 = dt.float32, dt.float32r, dt.bfloat16

    q_sb = qkvp.tile([128, T], f32r, tag="q", name=f"q_{h}")
    k_sb = qkvp.tile([128, T], f32r, tag="k", name=f"k_{h}")
    v_sb = qkvp.tile([128, T], bf16, tag="v", name=f"v_{h}")
    vT = qkvp.tile([128, T], bf16, tag="vT", name=f"vT_{h}")
    rstdT = qkvp.tile([128, 16], f32, tag="rT", name=f"rstdT_{h}")

    # ---- 2b: v = (Wv_h^T x)^T via transposes; bf16 token-major ----
    with tc.tile_pool(name=f"psv_{h}", bufs=1, space="PSUM") as psv, \
         tc.tile_pool(name=f"pst_{h}", bufs=2, space="PSUM") as pst:
        vps = {}
        for J in range(NT):
            vp = psv.tile([128, 512], f32, tag=f"v{J}", name=f"vp_{h}_{J}")
            vps[J] = vp
        for i in range(NCT):
            wt = wst.tile([128, 128], f32r, tag="w", name=f"wv_{h}_{i}")
            nc.sync.dma_start(wt[:],
                              env["wv_d"][i * 128:(i + 1) * 128,
                                          h * 128:(h + 1) * 128])
            for J in range(NT):
                nc.tensor.matmul(
                    vps[J][:], wt[:],
                    xts[:, i * T + J * 512:i * T + J * 512 + 512],
                    start=(i == 0), stop=(i == NCT - 1))
        for J in range(NT):
            nc.scalar.copy(vT[:, J * 512:(J + 1) * 512], vps[J][:])
        for tq in range(4):
            tp = pst.tile([128, 512], bf16, tag="tp", name=f"tp_{h}_{tq}")
            for k in range(4):
                tt = tq * 4 + k
                nc.tensor.transpose(tp[:, k * 128:(k + 1) * 128],
                                    vT[:, tt * 128:(tt + 1) * 128],
                                    env["idb"][:])
            nc.scalar.copy(v_sb[:, tq * 512:(tq + 1) * 512], tp[:])

    # ---- 2a: q/k projections, rotate (fused), ssq, rstd ----
    with tc.tile_pool(name=f"psa_{h}", bufs=1, space="PSUM") as psa, \
         tc.tile_pool(name=f"pss_{h}", bufs=1, space="PSUM") as pss:
        ssqps = pss.tile([8, 512], f32, name=f"ssqps_{h}")
        nsite = 0
        for wi, (w_d, wcol, dest) in enumerate(
                ((env["wq_d"], env["wcolq"], q_sb),
                 (env["wk_d"], env["wcolk"], k_sb))):
            qps = {}
            for J in range(NT):
                qp = psa.tile([128, 512], f32, tag=f"a{J}",
                              name=f"qp_{h}_{wi}_{J}")
                qps[J] = qp
            for i in range(NCT):
                wt = wst.tile([128, 128], f32r, tag="w", name=f"wt_{h}_{wi}_{i}")
                nc.sync.dma_start(wt[:],
                                  w_d[i * 128:(i + 1) * 128,
                                      h * 128:(h + 1) * 128])
                for J in range(NT):
                    nc.tensor.matmul(
                        qps[J][:], wt[:],
                        xts[:, i * T + J * 512:i * T + J * 512 + 512],
                        start=(i == 0), stop=(i == NCT - 1))
            for J in range(NT):
                site = (1 - wi) * 4 + J  # k sites rows 0-3, q rows 4-7
                qtmp = wkp.tile([128, 512], f32, tag=f"qt{J % 2}",
                                name=f"qtmp_{h}_{site}")
                nc.scalar.copy(qtmp[:], qps[J][:])  # frees bank a{J}
                sq = wkp.tile([128, 512], f32r, tag="sq", name=f"sq_{h}_{site}")
                nc.scalar.activation(sq[:], qtmp[:], AF.Square)
                nc.tensor.matmul(ssqps[:], wcol[:, 7 - site:15 - site], sq[:],
                                 start=(nsite == 0), stop=(nsite == 7))
                nsite += 1
                sl = slice(J * 512, (J + 1) * 512)
                # rot1 = q1 cos + q2 sin ; rot2 = q2 cos - q1 sin.
                # DVE inputs must share a start partition; the cross-half
                # sin products ride the (legal) shifted writes of tsw.
                tcc = wkp.tile([128, 512], f32, tag="tcc", name=f"tcc_{h}_{site}")
                tsw = wkp.tile([128, 512], f32, tag="tss", name=f"tsw_{h}_{site}")
                nc.vector.tensor_tensor(tcc[:], qtmp[:], env["trigc"][:, sl],
                                        op=ALU.mult)
                nc.vector.tensor_tensor(tsw[0:DH, :], qtmp[DH:128, :],
                                        env["trigs"][DH:128, sl], op=ALU.mult)
                nc.vector.tensor_tensor(tsw[DH:128, :], qtmp[0:DH, :],
                                        env["trigs"][0:DH, sl], op=ALU.mult)
                nc.vector.tensor_add(dest[0:DH, sl], tcc[0:DH, :],
                                     tsw[0:DH, :])
                nc.gpsimd.tensor_sub(dest[DH:128, sl], tcc[DH:128, :],
                                     tsw[DH:128, :])
        # batched rstd: k rows 0-3 get ln(SCL) bias; q rows 4-7 plain
        lnt = wkp.tile([8, 512], f32, tag="sq", name=f"lnt_{h}")
        nc.scalar.activation(lnt[:], ssqps[:], AF.Ln, scale=1.0 / 128.0,
                             bias=env["eps8"][:])
        rstd = wkp.tile([8, 512], f32r, tag="rstd8", name=f"rstd_{h}")
        nc.scalar.activation(rstd[:], lnt[:], AF.Exp, scale=-0.5,
                             bias=env["bias8"][:])
        # transpose k rows into per-key columns for the exp scale
        with tc.tile_pool(name=f"psr_{h}", bufs=1, space="PSUM") as psr:
            rt = psr.tile([128, 16], f32r, name=f"rt_{h}")
            for c in range(4):
                nc.tensor.transpose(rt[:, c * 4:(c + 1) * 4],
                                    rstd[0:4, c * 128:(c + 1) * 128],
                                    env["idr"][0:4, 0:4])
            nc.vector.tensor_copy(rstdT[:], rt[:])

    # ---- deferred q normalize (broadcast rstd over partitions) ----
    with tc.tile_pool(name=f"psn_{h}", bufs=2, space="PSUM") as psn:
        for J in range(NT):
            rrow = env["rowsp"].tile([1, 512], f32r, tag="r0",
                                     name=f"rrow_{h}_{J}")
            nc.sync.dma_start(rrow[:], rstd[4 + J:5 + J, :])
            rbps = psn.tile([128, 512], f32, tag="nb", name=f"rb_{h}_{J}")
            nc.tensor.matmul(rbps[:], env["onesB"][:], rrow[:],
                             start=True, stop=True)
            sl = slice(J * 512, (J + 1) * 512)
            nc.vector.tensor_tensor(q_sb[:, sl], q_sb[:, sl], rbps[:],
                                    op=ALU.mult)
    return q_sb, k_sb, v_sb, rstdT


def _head_attn(env, h, qkv, yall):
    """2c: causal attention for head h (transposed scores, pipelined)."""
    nc = env["nc"]
    tc = env["tc"]
    q_sb, k_sb, v_sb, rstdT = qkv
    expool, wkp, rowsp = env["expool"], env["wkp"], env["rowsp"]
    masks, onesAb, onesB = env["masks"], env["onesAb"], env["onesB"]
    f32, f32r, bf16 = dt.float32, dt.float32r, dt.bfloat16

    with tc.tile_pool(name=f"psc_{h}", bufs=1, space="PSUM") as psc:
        pend = [None]

        def epilogue(yps, dps, J):
            rcf = rowsp.tile([1, 512], f32, tag="rc", name=f"rcf_{h}_{J}")
            nc.vector.reciprocal_approx_fast(out=rcf[:], in_=dps[:])
            recip = rowsp.tile([1, 512], f32r, tag="rp", name=f"recip_{h}_{J}")
            nc.vector.tensor_copy(recip[:], rcf[:])
            rbps = psc.tile([128, 512], f32, tag="rb", bufs=1,
                            name=f"rbps_{h}_{J}")
            nc.tensor.matmul(rbps[:], onesB[:], recip[:], start=True, stop=True)
            rbsb = wkp.tile([128, 512], f32, tag="rbsb", name=f"rbsb_{h}_{J}")
            nc.vector.tensor_copy(rbsb[:], rbps[:])
            sl = slice(J * 512, (J + 1) * 512)
            if yall is None:
                yt = expool.tile([128, 512], bf16, tag="yt", name=f"yt_{h}_{J}")
                nc.vector.tensor_tensor(yt[:], yps[:], rbsb[:], op=ALU.mult)
                nc.sync.dma_start(
                    env["yspill"][:, h * T + J * 512:h * T + (J + 1) * 512],
                    yt[:])
            else:
                nc.vector.tensor_tensor(yall[:, 7 * T + J * 512:
                                             7 * T + (J + 1) * 512],
                                        yps[:], rbsb[:], op=ALU.mult)

        for J in range(NT):
            nI = 4 * J + 4
            yps = psc.tile([128, 512], f32, tag="y", bufs=2,
                           name=f"yps_{h}_{J}")
            dps = psc.tile([1, 512], f32, tag="d", bufs=2,
                           name=f"dps_{h}_{J}")
            uses = []
            pairs = []

            def emit_y(I, yps=yps, dps=dps, nI=nI, uses=uses, pairs=pairs):
                nc.tensor.matmul(
                    yps[:], v_sb[:, I * 128:(I + 1) * 128], uses[I][:],
                    start=(I == 0), stop=(I == nI - 1))
                if I % 2 == 1:
                    m = I // 2
                    nc.tensor.matmul(dps[:], onesAb[:], pairs[m][:],
                                     start=(m == 0), stop=(m == nI // 2 - 1))

            for I in range(nI):
                sps = psc.tile([128, 512], f32, tag="s", bufs=3,
                               name=f"sps_{h}_{J}_{I}")
                nc.tensor.matmul(
                    sps[:], k_sb[:, I * 128:(I + 1) * 128],
                    q_sb[:, J * 512:(J + 1) * 512], start=True, stop=True)
                ex = expool.tile([128, 512], bf16, tag=f"ex{I % 3}",
                                 name=f"ex_{h}_{J}_{I}")
                col = (I % 4) * 4 + (I // 4)
                nc.scalar.activation(ex[:], sps[:], AF.Exp,
                                     scale=rstdT[:, col:col + 1])
                if I >= 4 * J:
                    r = I - 4 * J
                    exm = expool.tile([128, 512], bf16, tag=f"mk{I % 3}",
                                      name=f"exm_{h}_{J}_{I}")
                    nc.vector.tensor_tensor(
                        exm[:], ex[:], masks[:, r * 512:(r + 1) * 512],
                        op=ALU.mult)
                    uses.append(exm)
                else:
                    uses.append(ex)
                if I % 2 == 1:
                    pm = expool.tile([128, 512], bf16, tag=f"ps{(I // 2) % 2}",
                                     name=f"pm_{h}_{J}_{I}")
                    nc.vector.tensor_add(pm[:], uses[I - 1][:], uses[I][:])
                    pairs.append(pm)
                if I == 1 and pend[0] is not None:
                    pend[0]()
                    pend[0] = None
                if I >= 2:
                    emit_y(I - 2)
            emit_y(nI - 2)
            emit_y(nI - 1)
            pend[0] = (lambda yps=yps, dps=dps, J=J: epilogue(yps, dps, J))
        pend[0]()
        pend[0] = None


def _host_prep(inputs):
    x = np.asarray(inputs["x"], dtype=np.float32)
    Wq = np.asarray(inputs["Wq"], dtype=np.float32)
    Wk = np.asarray(inputs["Wk"], dtype=np.float32)
    Wv = np.asarray(inputs["Wv"], dtype=np.float32)
    Wo = np.asarray(inputs["Wo"], dtype=np.float32)
    w_omega = np.asarray(inputs["w_omega"], dtype=np.float32)
    b_omega = np.asarray(inputs["b_omega"], dtype=np.float32)
    log_freq = np.asarray(inputs["log_freq"], dtype=np.float32)
    q_gamma = np.asarray(inputs["q_gamma"], dtype=np.float32)
    k_gamma = np.asarray(inputs["k_gamma"], dtype=np.float32)

    # host trig path (tiny): omega -> phi -> cos/sin tables per batch
    z = (x.reshape(B * T, C) @ w_omega.reshape(C).astype(np.float64)
         ).reshape(B, T) + float(b_omega[0])
    omega = 1.0 / (1.0 + np.exp(-z.astype(np.float64) / OMEGA_SCALE))
    phi = np.cumsum(omega, axis=1) - omega                       # (B,T)
    freq = np.exp(log_freq.astype(np.float64))                   # (DH,)
    ang = phi[:, None, :] * freq[:, None]                        # (B,DH,T)
    cosb = np.cos(ang).astype(np.float32)
    sinb = np.sin(ang).astype(np.float32)
    trigc = np.concatenate([cosb, cosb], axis=1)                 # (B,128,T)
    trigs = np.concatenate([sinb, sinb], axis=1)

    g2 = (q_gamma * k_gamma).astype(np.float32)                  # (128,)
    wk_scale = np.tile(g2, HG)                                   # (GD per group)
    inv_g2sq = np.zeros(128, dtype=np.float32)
    nz = np.abs(g2) > 1e-12
    inv_g2sq[nz] = 1.0 / (g2[nz] * g2[nz])

    p = np.arange(128)[:, None]
    c = np.arange(512)[None, :]
    masks = np.concatenate(
        [((p + r * 128) <= c).astype(np.float32) for r in range(4)], axis=1
    ).astype(ml_dtypes.bfloat16)
    onesAb = np.ones((128, 1), dtype=ml_dtypes.bfloat16)
    onesB = np.ones((1, 128), dtype=np.float32)
    idb = np.eye(128, dtype=ml_dtypes.bfloat16)
    idr = np.eye(128, dtype=np.float32)
    wcolq = np.zeros((128, 15), dtype=np.float32)
    wcolq[:, 7] = 1.0
    wcolk = np.zeros((128, 15), dtype=np.float32)
    wcolk[:, 7] = inv_g2sq
    bias8 = np.zeros((8, 1), dtype=np.float32)
    bias8[0:4] = math.log(SCL)  # k sites occupy rows 0-3

    in_maps = []
    for core in range(8):
        b, g = core // 2, core % 2
        wk_g = Wk[g * GD:(g + 1) * GD, :] * wk_scale[:, None]
        in_maps.append({
            "xt": _round_f32r(x[b].T),
            "wq": _round_f32r(Wq[g * GD:(g + 1) * GD, :].T),
            "wk": _round_f32r(wk_g.T),
            "wv": _round_f32r(Wv[g * GD:(g + 1) * GD, :].T),
            "wo": Wo[:, g * GD:(g + 1) * GD].T.astype(ml_dtypes.bfloat16),
            "trigc": trigc[b], "trigs": trigs[b],
            "masks": masks, "onesAb": onesAb, "onesB": _round_f32r(onesB),
            "idb": idb, "idr": _round_f32r(idr),
            "wcolq": _round_f32r(wcolq), "wcolk": _round_f32r(wcolk),
            "bias8": bias8,
        })
    return in_maps


def kernel(**inputs) -> np.ndarray:
    if "nc" not in _CACHE:
        _CACHE["nc"] = _build()
    nc = _CACHE["nc"]
    in_maps = _host_prep(inputs)
    res = run_bass_kernel_spmd(nc, in_maps, core_ids=list(range(8)))
    out = np.empty((B, T, C), dtype=np.float32)
    for b in range(B):
        out[b] = res.results[2 * b]["out"] + res.results[2 * b + 1]["out"]
    return out
